# revision 30
# baseline (speedup 1.0000x reference)
"""Trainium2 Bass kernel for nn_DiscriminatorAD (2-layer GCN discriminator).

Math (reference):
    h      = relu(adj @ (x @ W1) + b1)          # [N, 5]
    s      = (adj @ (h @ W2) + b2)              # [N]
    logits = s @ lin_W.T + lin_b                # [1, 1]
    out    = sigmoid(logits)

Key factorization: the output is a single scalar, so
    logits = u . q + b2 * sum(lin_W) + lin_b
where q = h @ W2 and u = lin_W @ adj.  Both contractions stream the SAME
elements of adj, so the device reads adj exactly ONCE.

Sharding: row-shard adj across 8 cores (1250 rows each).  Core c gets
A'_T = (diag(w) @ adj[rows_c, :]).T in bf16 — the transposed shard with
lin_W pre-folded into the rows (w clamped away from 0 so it can be
divided back out) — relaid out on the host so that each SBUF partition's
data for a GROUP of 6 column-chunks is contiguous in DRAM (128 large
descriptors per group DMA instead of 768; HWDGE descriptor generation
at ~5ns/descriptor was the previous bottleneck).

Per 128-column chunk k of A'_T (j = adj column on partitions, i = the
core's own rows on the free axis):
  - u-pass: sum over the free axis gives u[jchunk] = sum_i w_i*adj[i,j]
    directly.  Groups alternate between VectorE (one fused [128,6,1250]
    tensor_reduce) and ScalarE (activation-Copy with accum_out), which
    run concurrently; both engines stream ~1 elem/lane/cycle.
  - h-pass (TensorE): lhsT = S1[jchunk] ([128,5] stationary), rhs =
    chunk slice -> accumulates w_i-scaled h^T in PSUM over all chunks.
The w_i scale is divided back out of h^T with one tiny [5,1250]
multiply before the relu(+b1), then q^T = W2^T @ relu_h^T.  Outputs per
core: u partial [128,79] and q rows [1,1250]; the host combines them
into the scalar logits.  bf16 is safe: logits ~ -374000, bf16 moves it
~1e-4 relative, and float32 sigmoid underflows to exactly 0.0 either
way (saturates for |logits| > ~104).
"""

import numpy as np
import ml_dtypes

N = 10000
NCORES = 8
ROWS = N // NCORES           # 1250 rows of adj per core
KCH = (N + 127) // 128       # 79 column chunks (78 full + 16-row tail)
# Variable DMA group sizes (in 128-column chunks): small groups at the
# start so compute begins ~2us in (concurrent big first-DMAs would delay
# the first arrival to ~20us), big groups in the middle for descriptor
# efficiency, small groups at the end so the final reduce is short.
GROUPS = [1, 1, 2, 2, 4] + [6] * 10 + [4, 3, 1]   # sums to 78
GMAX = max(GROUPS)
TAILP = N - (KCH - 1) * 128  # partitions in the tail chunk (16)
F1, F2 = 512, 1024           # h^T free-dim splits (PSUM bank = 512 fp32)
W_EPS = 1e-6                 # |lin_W| clamp so 1/w is finite

_compiled = None


def _build():
    """Build the SPMD Bass program once; returns nc."""
    from contextlib import ExitStack

    import concourse.bacc as bacc
    import concourse.mybir as mybir
    import concourse.tile as tile

    nc = bacc.Bacc("TRN2", target_bir_lowering=False, debug=False)

    bf16 = mybir.dt.bfloat16
    f32 = mybir.dt.float32

    atg = nc.dram_tensor("atg", [(KCH - 1) * 128, ROWS], bf16, kind="ExternalInput").ap()
    att = nc.dram_tensor("att", [TAILP, ROWS], bf16, kind="ExternalInput").ap()
    s1p = nc.dram_tensor("s1p", [128, KCH * 5], bf16, kind="ExternalInput").ap()
    winv = nc.dram_tensor("winv", [5, ROWS], f32, kind="ExternalInput").ap()
    b1 = nc.dram_tensor("b1", [5, 1], f32, kind="ExternalInput").ap()
    w2 = nc.dram_tensor("w2", [5, 1], bf16, kind="ExternalInput").ap()
    u_out = nc.dram_tensor("u_out", [128, KCH], f32, kind="ExternalOutput").ap()
    q_out = nc.dram_tensor("q_out", [1, ROWS], f32, kind="ExternalOutput").ap()

    with tile.TileContext(nc) as tc, ExitStack() as ctx:
        consts = ctx.enter_context(tc.tile_pool(name="consts", bufs=1))
        strips = ctx.enter_context(tc.tile_pool(name="strips", bufs=4))
        psum = ctx.enter_context(tc.tile_pool(name="psum", bufs=1, space="PSUM"))
        small = ctx.enter_context(tc.tile_pool(name="small", bufs=1))

        s1p_sb = consts.tile([128, KCH * 5], bf16)
        nc.sync.dma_start(s1p_sb[:], s1p[:])
        winv_sb = consts.tile([5, ROWS], f32)
        nc.sync.dma_start(winv_sb[:], winv[:])
        b1_sb = consts.tile([5, 1], f32)
        nc.sync.dma_start(b1_sb[:], b1[:])
        w2_sb = consts.tile([5, 1], bf16)
        nc.sync.dma_start(w2_sb[:], w2[:])

        u_sb = small.tile([128, KCH], f32)
        scratch = small.tile([128, ROWS], bf16)

        # h^T accumulators: [5, 1250] split across three PSUM banks
        hp0 = psum.tile([5, F1], f32)
        hp1 = psum.tile([5, F2 - F1], f32)
        hp2 = psum.tile([5, ROWS - F2], f32)

        def do_matmuls(k, tile_, col0, kp):
            lhsT = s1p_sb[:kp, k * 5 : (k + 1) * 5]
            # processed tail-first, then chunks 0..77 in order
            st, sp = (k == KCH - 1), (k == KCH - 2)
            c = col0
            nc.tensor.matmul(hp0[:], lhsT, tile_[:kp, c : c + F1], start=st, stop=sp)
            nc.tensor.matmul(hp1[:], lhsT, tile_[:kp, c + F1 : c + F2], start=st, stop=sp)
            nc.tensor.matmul(hp2[:], lhsT, tile_[:kp, c + F2 : c + ROWS], start=st, stop=sp)

        copy_f = mybir.ActivationFunctionType.Copy

        # tail chunk first: its DMA is tiny so the PE starts immediately,
        # and it carries the start=True accumulation flag.
        tail = strips.tile([128, GMAX * ROWS], bf16)
        nc.sync.dma_start(tail[:TAILP, 0:ROWS], att[:])
        do_matmuls(KCH - 1, tail, 0, TAILP)
        nc.scalar.activation(
            scratch[:TAILP, :], tail[:TAILP, 0:ROWS], copy_f,
            accum_out=u_sb[:TAILP, KCH - 1 : KCH],
        )

        # cost-balanced engine alternation for the u-reduce
        dve_cost, act_cost = 0.0, 1.4  # ACT opens with the tail chunk
        k0 = 0
        row_off = 0
        for sz in GROUPS:
            gt = strips.tile([128, GMAX * ROWS], bf16)
            src = atg[row_off : row_off + 128 * sz, :].rearrange(
                "(p r) i -> p (r i)", r=sz
            )
            nc.sync.dma_start(gt[:, 0 : sz * ROWS], src)
            for g in range(sz):
                do_matmuls(k0 + g, gt, g * ROWS, 128)
            if dve_cost <= act_cost:
                nc.vector.tensor_reduce(
                    u_sb[:, k0 : k0 + sz],
                    gt[:, 0 : sz * ROWS].rearrange("p (g i) -> p g i", g=sz),
                    axis=mybir.AxisListType.X,
                    op=mybir.AluOpType.add,
                )
                dve_cost += 1.35 * sz
            else:
                for g in range(sz):
                    nc.scalar.activation(
                        scratch[:], gt[:, g * ROWS : (g + 1) * ROWS], copy_f,
                        accum_out=u_sb[:, k0 + g : k0 + g + 1],
                    )
                act_cost += 1.37 * sz
            k0 += sz
            row_off += 128 * sz

        # undo the w_i scaling folded into A'_T, then h = relu(. + b1)
        t_sb = small.tile([5, ROWS], f32)
        nc.vector.tensor_tensor(t_sb[:, 0:F1], hp0[:], winv_sb[:, 0:F1], op=mybir.AluOpType.mult)
        nc.vector.tensor_tensor(t_sb[:, F1:F2], hp1[:], winv_sb[:, F1:F2], op=mybir.AluOpType.mult)
        nc.vector.tensor_tensor(t_sb[:, F2:ROWS], hp2[:], winv_sb[:, F2:ROWS], op=mybir.AluOpType.mult)
        h_sb = small.tile([5, ROWS], bf16)
        relu = mybir.ActivationFunctionType.Relu
        nc.scalar.activation(h_sb[:], t_sb[:], relu, bias=b1_sb[:])

        # q^T = W2^T @ h^T   ([1, 1250])
        qp0 = psum.tile([1, F1], f32)
        qp1 = psum.tile([1, F2 - F1], f32)
        qp2 = psum.tile([1, ROWS - F2], f32)
        nc.tensor.matmul(qp0[:], w2_sb[:], h_sb[:, 0:F1], start=True, stop=True)
        nc.tensor.matmul(qp1[:], w2_sb[:], h_sb[:, F1:F2], start=True, stop=True)
        nc.tensor.matmul(qp2[:], w2_sb[:], h_sb[:, F2:ROWS], start=True, stop=True)
        q_sb = small.tile([1, ROWS], f32)
        nc.scalar.copy(q_sb[:, 0:F1], qp0[:])
        nc.scalar.copy(q_sb[:, F1:F2], qp1[:])
        nc.scalar.copy(q_sb[:, F2:ROWS], qp2[:])

        nc.sync.dma_start(u_out[:], u_sb[:])
        nc.sync.dma_start(q_out[:], q_sb[:])

    nc.compile()
    return nc


def _get_compiled():
    global _compiled
    if _compiled is None:
        _compiled = _build()
    return _compiled


def _prepare_inputs(x, adj, W1, b1, W2, lin_W):
    """Host-side shard prep: returns per-core in_maps."""
    bf16 = ml_dtypes.bfloat16
    s1 = (x.astype(np.float32) @ W1.astype(np.float32)).astype(bf16)  # [N, 5]
    # s1 packed as [128, KCH*5]: s1p[p, k*5+c] = s1[k*128+p, c]
    s1_pad = np.zeros((KCH * 128, 5), dtype=bf16)
    s1_pad[:N] = s1
    s1p = np.ascontiguousarray(
        s1_pad.reshape(KCH, 128, 5).transpose(1, 0, 2).reshape(128, KCH * 5)
    )
    b1_in = b1.reshape(5, 1).astype(np.float32)
    w2_in = W2.reshape(5, 1).astype(bf16)

    lw = lin_W.reshape(-1).astype(np.float64)
    w_safe = np.where(np.abs(lw) < W_EPS, np.where(lw < 0, -W_EPS, W_EPS), lw)

    in_maps = []
    for c in range(NCORES):
        r0 = c * ROWS
        ws = w_safe[r0 : r0 + ROWS]
        # A'_T[j, i] = adj[r0+i, j] * w_safe[r0+i]  (fold lin_W into rows)
        at_c = (adj[r0 : r0 + ROWS, :] * ws[:, None]).astype(bf16).T  # [N, ROWS]
        # group layout: per group of sz chunks, partition p's data for all
        # sz chunks is contiguous: block[p, g, i] = A'_T[(k0+g)*128 + p, i]
        blocks = []
        k0 = 0
        for sz in GROUPS:
            blk = (
                np.asarray(at_c[k0 * 128 : (k0 + sz) * 128])
                .reshape(sz, 128, ROWS)
                .transpose(1, 0, 2)
                .reshape(128 * sz, ROWS)
            )
            blocks.append(blk)
            k0 += sz
        atg_c = np.ascontiguousarray(np.concatenate(blocks, axis=0))
        att_c = np.ascontiguousarray(np.asarray(at_c[(KCH - 1) * 128 :]))
        # 1 / (w as seen by the device): bf16(w) is what actually scaled
        # the matmul inputs, so invert the bf16-rounded value.
        ws_dev = ws.astype(bf16).astype(np.float64)
        winv_c = np.ascontiguousarray(
            np.broadcast_to((1.0 / ws_dev).astype(np.float32), (5, ROWS))
        )
        in_maps.append(
            {"atg": atg_c, "att": att_c, "s1p": s1p, "winv": winv_c,
             "b1": b1_in, "w2": w2_in}
        )
    return in_maps


def kernel(x, adj, W1, b1, W2, b2, lin_W, lin_b):
    from concourse.bass_utils import run_bass_kernel_spmd

    x = np.asarray(x)
    adj = np.asarray(adj)
    W1 = np.asarray(W1)
    b1 = np.asarray(b1)
    W2 = np.asarray(W2)
    b2 = np.asarray(b2)
    lin_W = np.asarray(lin_W)
    lin_b = np.asarray(lin_b)

    nc = _get_compiled()
    in_maps = _prepare_inputs(x, adj, W1, b1, W2, lin_W)
    res = run_bass_kernel_spmd(nc, in_maps, list(range(NCORES)))

    # host combine: u_full = sum_c u_c ; q_full = concat_c q_c
    u_full = np.zeros(N, dtype=np.float64)
    q_full = np.zeros(N, dtype=np.float64)
    for c in range(NCORES):
        u_c = res.results[c]["u_out"]  # [128, KCH]
        q_c = res.results[c]["q_out"]  # [1, ROWS]
        u_full += u_c.T.reshape(-1)[:N].astype(np.float64)
        q_full[c * ROWS : (c + 1) * ROWS] = q_c.reshape(-1).astype(np.float64)

    logits = (
        float(u_full @ q_full)
        + float(b2.astype(np.float64).sum()) * float(lin_W.astype(np.float64).sum())
        + float(lin_b.astype(np.float64).reshape(-1)[0])
    )
    # float32 sigmoid, numerically stable (saturates to exactly 0.0 / 1.0)
    lg = np.float32(logits)
    if lg >= 0:
        out = np.float32(1.0) / (np.float32(1.0) + np.exp(-lg, dtype=np.float32))
    else:
        e = np.exp(lg, dtype=np.float32)
        out = e / (np.float32(1.0) + e)
    return np.array([[out]], dtype=np.float32)
